# revision 42
# baseline (speedup 1.0000x reference)
"""Trainium2 Bass kernel for nn_Attention_63934883168998.

Math (per token t): q,k,v = x W{q,k,v}^T reshaped (16 heads, 64); scores over
HEADS: S = q k^T / 8 (16x16), A = softmax(S), out = A v -> (1024); y = out Wo^T.

Sharding: pure data parallel over the 16384 tokens -> 2048 tokens/core.
All on-chip data fp16 (PE fp16 matmul = full rate).

Structure (v10): 256-token pairs; projections emitted ONE PAIR AHEAD of the
DVE score chain (the product never waits on q/k PSUM->SBUF copies); the
AV/output chain runs one pair behind; scatter DMAs merged across the pair
(320B runs) and issued only on the two HWDGE queues (sync/scalar); GpSimd
completely unused (its SBUF port is shared with the Vector engine -- GpSimd
work slows DVE ~1:1, and small-run DMA traffic does too); the block-diagonal
AV stationary is built by 8 partition-sliced sbuf->sbuf DMAs into statically
zeroed buffers.  The Vector engine runs only the score product, 6-level tree,
softmax sum + reciprocal and A-normalize, back-to-back with ~0 idle.
"""

import numpy as np
import ml_dtypes

N_CORES = 8
HID = 1024
NH, HD = 16, 64
TILE = 128
PAIR = 256
TPC = 16384 // N_CORES      # tokens per core
NP = TPC // PAIR            # tile-pairs per core (8)
NCH = HID // 128            # 8 hidden chunks
NG = TILE // 8              # 16 groups of 8 tokens

_cache = {}


def _build():
    if "nc" in _cache:
        return
    import concourse.bacc as bacc
    import concourse.mybir as mybir
    from concourse import tile

    f16 = mybir.dt.float16
    f32 = mybir.dt.float32
    AX = mybir.AxisListType
    OP = mybir.AluOpType
    AF = mybir.ActivationFunctionType

    nc = bacc.Bacc("TRN2", target_bir_lowering=False, debug=False)
    xt = nc.dram_tensor("xt", (HID, TPC), f16, kind="ExternalInput").ap()
    wts = {
        n: nc.dram_tensor(n, (HID, HID), f16, kind="ExternalInput").ap()
        for n in ("wqt", "wkt", "wvt", "wot")
    }
    ident_d = nc.dram_tensor("ident", (128, 128), f16, kind="ExternalInput").ap()
    y = nc.dram_tensor("y", (TPC, HID), f16, kind="ExternalOutput").ap()

    with tile.TileContext(nc) as tc:
        with (
            tc.tile_pool(name="wpool", bufs=1) as wpool,
            tc.tile_pool(name="io", bufs=2) as iop,
            tc.tile_pool(name="qk", bufs=2) as qkp,
            tc.tile_pool(name="sc", bufs=1) as scp,
            tc.tile_pool(name="av", bufs=2) as avp,
            tc.tile_pool(name="av1", bufs=1) as avp1,
            tc.tile_pool(name="st1", bufs=1) as st1,
            tc.tile_pool(name="psum", bufs=1, space="PSUM") as pp,
        ):
            # ---- resident weights / constants ----
            w_sb = {}
            for n in ("wqt", "wkt", "wvt", "wot"):
                w_sb[n] = wpool.tile([128, NCH, HID], f16, name=n + "_sb", tag=n)
            xt_r = xt.rearrange("(c p) t -> p c t", p=128)

            def load_xT(i):
                t = iop.tile([128, NCH, PAIR], f16, name=f"xT{i}", tag="xT")
                if i == 0:
                    # startup-critical: spread across 8 DMA engines
                    for c in range(NCH):
                        eng = (nc.sync, nc.scalar)[c % 2]
                        eng.dma_start(
                            t[:, c, :], xt_r[:, c, i * PAIR : (i + 1) * PAIR]
                        )
                else:
                    nc.sync.dma_start(t[:], xt_r[:, :, i * PAIR : (i + 1) * PAIR])
                return t

            xT_bufs = {0: load_xT(0)}
            k = 0
            wengs = [nc.scalar, nc.sync]
            worder = [(n, c) for c in range(NCH) for n in ("wqt", "wkt")]
            worder += [("wvt", c) for c in range(NCH)]
            worder += [("wot", c) for c in range(NCH)]
            for n, c in worder:
                wengs[k % 2].dma_start(
                    w_sb[n][:, c, :], wts[n][c * 128 : (c + 1) * 128, :]
                )
                k += 1
            ident_sb = wpool.tile([128, 128], f16, tag="ident")
            nc.scalar.dma_start(ident_sb[:], ident_d[:])
            # two static block-diag stationary buffers, zeroed once; per pair
            # only the 8 diagonal (b'=b) blocks are rewritten via DMA
            abd_bufs = []
            for j in range(2):
                ab = wpool.tile([128, NG, 8, NH, 2], f16, name=f"abds{j}",
                                tag=f"abds{j}")
                nc.vector.memset(ab[:], 0.0)
                abd_bufs.append(ab)

            # ---- per-pair state kept across pipeline stages ----
            state = {}

            def new_state(i):
                state[i] = {
                    "xT": xT_bufs.pop(i),
                    "q": {}, "k": {},
                    "comb2": iop.tile([128, NH, 80, 2], f16, name=f"c2_{i}",
                                      tag="comb2"),
                    "comb_k2": avp.tile([128, NG, 80, 2], f16, name=f"ck2_{i}",
                                        tag="comb_k2"),
                }

            def proj(i, T, which):
                """Project tile (i,T). which in ('qk', 'v')."""
                st = state[i]
                xT = st["xT"]
                if which == "qk":
                    ps_q = pp.tile([128, HID], f32, name=f"psq{i}_{T}", tag="psA")
                    ps_k = pp.tile([128, HID], f32, name=f"psk{i}_{T}", tag="psB")
                    for c in range(NCH):
                        stat = xT[:, c, T * TILE : (T + 1) * TILE]
                        for n, ps in (("wqt", ps_q), ("wkt", ps_k)):
                            for h in range(2):
                                nc.tensor.matmul(
                                    ps[:, h * 512 : (h + 1) * 512],
                                    stat,
                                    w_sb[n][:, c, h * 512 : (h + 1) * 512],
                                    start=(c == 0),
                                    stop=(c == NCH - 1),
                                )
                    q_sb = qkp.tile([128, NH, HD], f16, name=f"q{i}_{T}", tag=f"q{T}")
                    k_sb = qkp.tile([128, NH, HD], f16, name=f"k{i}_{T}", tag=f"k{T}")
                    nc.scalar.copy(q_sb[:].rearrange("p h d -> p (h d)"), ps_q[:])
                    nc.scalar.copy(k_sb[:].rearrange("p h d -> p (h d)"), ps_k[:])
                    st["q"][T] = q_sb
                    st["k"][T] = k_sb
                else:
                    ps_v = pp.tile([128, HID], f32, name=f"psv{i}_{T}", tag="psC")
                    for c in range(NCH):
                        stat = xT[:, c, T * TILE : (T + 1) * TILE]
                        for h in range(2):
                            nc.tensor.matmul(
                                ps_v[:, h * 512 : (h + 1) * 512],
                                stat,
                                w_sb["wvt"][:, c, h * 512 : (h + 1) * 512],
                                start=(c == 0),
                                stop=(c == NCH - 1),
                            )
                    # comb2[t, g, 0:16, T] = A (later); [t, g, 16:80, T] = v
                    nc.scalar.copy(
                        st["comb2"][:, :, 16:, T],
                        ps_v[:].rearrange("p (g d) -> p g d", g=NH),
                    )

            def prod_op(i, T):
                """DVE product for tile (i,T): prod[t, h, g, d] (h-major so the
                softmax g-sum reduces a contiguous inner axis)."""
                st = state[i]
                q_sb, k_sb = st["q"][T], st["k"][T]
                prod = st1.tile([128, NH, NH, HD], f16, name=f"pr{i}{T}", tag="prod")
                q_ap = q_sb[:].unsqueeze(2).broadcast_to((128, NH, NH, HD))
                k_ap = k_sb[:].unsqueeze(1).broadcast_to((128, NH, NH, HD))
                nc.vector.tensor_tensor(prod[:], k_ap, q_ap, op=OP.mult)
                st["prod"] = prod

            def tree(i, T):
                """DVE tree levels 1-2 over d; level-2 output parks in the
                pair-shared scrBp so levels 3-6 run pair-merged."""
                st = state[i]
                p3 = st["prod"][:].rearrange("p h g d -> p (h g) d")
                scrA = st1.tile([128, NH * NH, 32], f16, tag="scrA")
                if T == 0:
                    st["scrBp"] = st1.tile([128, 2, NH * NH, 16], f16,
                                           name=f"sbp_{i}", tag="scrBp")
                with nc.allow_low_precision(reason="fp16 score partials"):
                    nc.vector.tensor_tensor(
                        scrA[:], p3[:, :, 0:32], p3[:, :, 32:64], op=OP.add
                    )
                    nc.vector.tensor_tensor(
                        st["scrBp"][:, T, :, :], scrA[:, :, 0:16],
                        scrA[:, :, 16:32], op=OP.add,
                    )

            def tail5(i):
                """Pair-merged tree levels 3-6 (both tiles in one op each);
                late-level scratch aliases buffers that are free by then."""
                st = state[i]
                scrBp = st["scrBp"]
                scrA34 = st1.tile([128, 2, NH * NH, 8], f16, tag="scrA")
                scrB2 = st1.tile([128, 2, NH * NH, 4], f16, tag="scrBp")
                scrA5 = st1.tile([128, 2, NH * NH, 2], f16, tag="scrA")
                scores2 = scp.tile([128, 2, NH * NH], f16, name=f"s{i}",
                                   tag="scores2")
                with nc.allow_low_precision(reason="fp16 score partials"):
                    nc.vector.tensor_tensor(
                        scrA34[:], scrBp[:, :, :, 0:8], scrBp[:, :, :, 8:16],
                        op=OP.add,
                    )
                    nc.vector.tensor_tensor(
                        scrB2[:], scrA34[:, :, :, 0:4], scrA34[:, :, :, 4:8],
                        op=OP.add,
                    )
                    nc.vector.tensor_tensor(
                        scrA5[:], scrB2[:, :, :, 0:2], scrB2[:, :, :, 2:4],
                        op=OP.add,
                    )
                    nc.vector.tensor_tensor(
                        scores2[:].unsqueeze(3),
                        scrA5[:, :, :, 0:1],
                        scrA5[:, :, :, 1:2],
                        op=OP.add,
                    )
                st["scores2"] = scores2

            def exp_op(i):
                st = state[i]
                ex2 = scp.tile([128, 2, NH * NH], f16, name=f"ex{i}", tag="ex2")
                nc.scalar.activation(ex2[:], st["scores2"][:], AF.Exp, scale=0.125)
                st["ex2"] = ex2

            def softmax_tail(i):
                """Pair-merged DVE ssum-reduce + recip + A-normalize,
                positioned after the next product so nothing head-blocks."""
                st = state[i]
                ex2 = st["ex2"]
                ssum2 = scp.tile([128, 2, NH], f32, tag="ssum2")
                ex_hg = ex2[:].rearrange("p T (h g) -> p T h g", h=NH)
                nc.vector.tensor_reduce(ssum2[:], ex_hg, axis=AX.X, op=OP.add)
                rs2 = scp.tile([128, 2, NH], f32, tag="rs2")
                nc.vector.reciprocal(
                    rs2[:].rearrange("p T h -> p (T h)"),
                    ssum2[:].rearrange("p T h -> p (T h)"),
                )
                # A into comb2 A-slots (both tiles):
                # comb2[t, g, h, T] = ex2[t, T, (h,g)] * rs2[t, T, h]
                nc.vector.tensor_tensor(
                    st["comb2"][:, :, 0:16, :],
                    ex2[:].rearrange("p T (h g) -> p g h T", h=NH),
                    rs2[:]
                    .rearrange("p T h -> p h T")
                    .unsqueeze(1)
                    .broadcast_to((128, NH, NH, 2)),
                    op=OP.mult,
                )

            def comb_scatter(i):
                """comb2 -> comb_k2: (b,g)-partition layout, both tiles per DMA
                (320B contiguous runs)."""
                st = state[i]
                comb2, comb_k2 = st["comb2"], st["comb_k2"]
                for grp in range(NG):
                    eng = (nc.sync, nc.scalar)[grp % 2]
                    eng.dma_start(
                        comb_k2[:, grp, :, :],
                        comb2[grp * 8 : (grp + 1) * 8, :, :, :],
                    )

            def abd(i):
                """Overwrite the 8 diagonal blocks of the static block-diag
                stationary with this pair's A values (sbuf->sbuf DMAs; engines
                can't start at partition offset 16)."""
                st = state[i]
                ab = abd_bufs[i % 2]
                for b in range(8):
                    eng = (nc.sync, nc.scalar)[b % 2]
                    eng.dma_start(
                        ab[b * 16 : (b + 1) * 16, :, b, :, :],
                        st["comb_k2"][b * 16 : (b + 1) * 16, :, 0:16, :],
                    )
                st["abd2"] = ab

            def av(e, T):
                """AV matmuls for tile (e,T) -> attn_pm2[..., T]."""
                st = state[e]
                abd2, comb_k2 = st["abd2"], st["comb_k2"]
                if T == 0:
                    st["attn_pm2"] = avp1.tile(
                        [128, NH, HD, 2], f16, name=f"apm{e}", tag="attn_pm2"
                    )
                pa = [
                    pp.tile([128, NG // 2, HD], f32, name=f"pa{e}{T}{h}",
                            tag=f"pav{h}")
                    for h in range(2)
                ]
                for grp in range(NG):
                    nc.tensor.matmul(
                        pa[grp // 8][:, grp % 8, :],
                        abd2[:, grp, :, :, T].rearrange("p b h -> p (b h)"),
                        comb_k2[:, grp, 16:, T],
                        start=True,
                        stop=True,
                    )
                for h in range(2):
                    nc.scalar.copy(
                        st["attn_pm2"][:, h * 8 : (h + 1) * 8, :, T], pa[h][:]
                    )

            def attn_scatter(e):
                st = state[e]
                st["attn16_2"] = avp1.tile(
                    [128, NH, HD, 2], f16, name=f"a16{e}", tag="attn16_2"
                )
                for grp in range(NG):
                    eng = (nc.scalar, nc.sync)[grp % 2]
                    eng.dma_start(
                        st["attn16_2"][grp * 8 : (grp + 1) * 8, :, :, :],
                        st["attn_pm2"][:, grp, :, :],
                    )

            def trans_wo(e, T):
                """PE transposes + output projection for tile (e,T)."""
                st = state[e]
                if T == 0:
                    st["y_sb2"] = avp1.tile(
                        [128, 2, HID], f16, name=f"ysb{e}", tag="y_sb2"
                    )
                oT = avp1.tile([128, NCH, TILE], f16, name=f"oT{e}{T}", tag=f"oT{T}")
                for half in range(2):
                    pt = pp.tile([128, 512], f16, name=f"pt{e}{T}{half}",
                                 tag=f"pav{half}")
                    for j in range(4):
                        c = half * 4 + j
                        nc.tensor.transpose(
                            pt[:, j * 128 : (j + 1) * 128],
                            st["attn16_2"][:, 2 * c : 2 * c + 2, :, T].rearrange(
                                "p h d -> p (h d)"
                            ),
                            ident_sb[:],
                        )
                    nc.scalar.copy(
                        oT[:, half * 4 : (half + 1) * 4, :].rearrange(
                            "p c t -> p (c t)"
                        ),
                        pt[:],
                    )
                py = [
                    pp.tile([128, 512], f32, name=f"py{e}{T}{h}", tag=f"pav{h}")
                    for h in range(2)
                ]
                for c in range(NCH):
                    for h in range(2):
                        nc.tensor.matmul(
                            py[h][:],
                            oT[:, c, :],
                            w_sb["wot"][:, c, h * 512 : (h + 1) * 512],
                            start=(c == 0),
                            stop=(c == NCH - 1),
                        )
                for h in range(2):
                    nc.scalar.copy(
                        st["y_sb2"][:, T, h * 512 : (h + 1) * 512], py[h][:]
                    )

            def y_out(e):
                st = state[e]
                t0 = e * PAIR
                ydst = y[t0 : t0 + PAIR, :].rearrange("(T t) f -> t T f", T=2)
                if e == NP - 1:
                    # drain-critical: spread across 4 DMA engines
                    for j in range(4):
                        eng = (nc.sync, nc.scalar)[j % 2]
                        eng.dma_start(
                            ydst[:, :, j * 256 : (j + 1) * 256],
                            st["y_sb2"][:, :, j * 256 : (j + 1) * 256],
                        )
                else:
                    nc.sync.dma_start(ydst, st["y_sb2"][:])

            # ---- main loop: projections ONE pair ahead, AV one pair behind --
            # iteration i: proj(i+1), DVE scores(i), AV/output chain(i-1)
            new_state(0)
            proj(0, 0, "qk")
            proj(0, 0, "v")
            proj(0, 1, "qk")
            proj(0, 1, "v")
            for i in range(NP + 1):
                e = i - 1
                if i + 1 < NP:
                    xT_bufs[i + 1] = load_xT(i + 1)
                    new_state(i + 1)
                    proj(i + 1, 0, "qk")
                if i < NP:
                    prod_op(i, 0)
                if e >= 0:
                    softmax_tail(e)     # deferred: sits after prod(i,T0)
                    comb_scatter(e)
                    abd(e)
                if i + 1 < NP:
                    proj(i + 1, 0, "v")
                    proj(i + 1, 1, "qk")
                if i < NP:
                    tree(i, 0)
                if e >= 0:
                    av(e, 0)
                    av(e, 1)
                    attn_scatter(e)
                if i < NP:
                    prod_op(i, 1)
                if i + 1 < NP:
                    proj(i + 1, 1, "v")
                if e >= 0:
                    trans_wo(e, 0)
                    trans_wo(e, 1)
                    y_out(e)
                    del state[e]
                if i < NP:
                    tree(i, 1)
                    tail5(i)
                    exp_op(i)

    nc.compile()
    _cache["nc"] = nc


def _prep_inputs(x, wq, wk, wv, wo):
    x2 = np.asarray(x, dtype=np.float32).reshape(-1, HID)
    w16 = {
        n: np.ascontiguousarray(np.asarray(w, dtype=np.float32).T).astype(np.float16)
        for n, w in (("wqt", wq), ("wkt", wk), ("wvt", wv), ("wot", wo))
    }
    in_maps = []
    for i in range(N_CORES):
        sh = x2[i * TPC : (i + 1) * TPC].astype(np.float16)
        m = {"xt": np.ascontiguousarray(sh.T),
             "ident": np.eye(128, dtype=np.float16)}
        m.update(w16)
        in_maps.append(m)
    return in_maps


def kernel(x, wq, wk, wv, wo, _trace=False):
    from concourse import bass_utils

    _build()
    in_maps = _prep_inputs(x, wq, wk, wv, wo)
    res = bass_utils.run_bass_kernel_spmd(
        _cache["nc"], in_maps, core_ids=list(range(N_CORES)), trace=_trace
    )
    kernel.last_result = res
    B, S = 4, 4096
    out = np.concatenate([r["y"] for r in res.results], axis=0)
    return out.reshape(B, S, HID).astype(np.float32)


# revision 43
# speedup vs baseline: 1.0150x; 1.0150x over previous
"""Trainium2 Bass kernel for nn_Attention_63934883168998.

Math (per token t): q,k,v = x W{q,k,v}^T reshaped (16 heads, 64); scores over
HEADS: S = q k^T / 8 (16x16), A = softmax(S), out = A v -> (1024); y = out Wo^T.

Sharding: pure data parallel over the 16384 tokens -> 2048 tokens/core.
All on-chip data fp16 (PE fp16 matmul = full rate).

Structure (v10): 256-token pairs; projections emitted ONE PAIR AHEAD of the
DVE score chain (the product never waits on q/k PSUM->SBUF copies); the
AV/output chain runs one pair behind; scatter DMAs merged across the pair
(320B runs) and issued only on the two HWDGE queues (sync/scalar); GpSimd
completely unused (its SBUF port is shared with the Vector engine -- GpSimd
work slows DVE ~1:1, and small-run DMA traffic does too); the block-diagonal
AV stationary is built by 8 partition-sliced sbuf->sbuf DMAs into statically
zeroed buffers.  The Vector engine runs only the score product, 6-level tree,
softmax sum + reciprocal and A-normalize, back-to-back with ~0 idle.
"""

import numpy as np
import ml_dtypes

N_CORES = 8
HID = 1024
NH, HD = 16, 64
TILE = 128
PAIR = 256
TPC = 16384 // N_CORES      # tokens per core
NP = TPC // PAIR            # tile-pairs per core (8)
NCH = HID // 128            # 8 hidden chunks
NG = TILE // 8              # 16 groups of 8 tokens

_cache = {}


def _build():
    if "nc" in _cache:
        return
    import concourse.bacc as bacc
    import concourse.mybir as mybir
    from concourse import tile

    f16 = mybir.dt.float16
    f32 = mybir.dt.float32
    AX = mybir.AxisListType
    OP = mybir.AluOpType
    AF = mybir.ActivationFunctionType

    nc = bacc.Bacc("TRN2", target_bir_lowering=False, debug=False)
    xt = nc.dram_tensor("xt", (HID, TPC), f16, kind="ExternalInput").ap()
    wts = {
        n: nc.dram_tensor(n, (HID, HID), f16, kind="ExternalInput").ap()
        for n in ("wqt", "wkt", "wvt", "wot")
    }
    ident_d = nc.dram_tensor("ident", (128, 128), f16, kind="ExternalInput").ap()
    y = nc.dram_tensor("y", (TPC, HID), f16, kind="ExternalOutput").ap()

    with tile.TileContext(nc) as tc:
        with (
            tc.tile_pool(name="wpool", bufs=1) as wpool,
            tc.tile_pool(name="io", bufs=2) as iop,
            tc.tile_pool(name="qk", bufs=2) as qkp,
            tc.tile_pool(name="sc", bufs=1) as scp,
            tc.tile_pool(name="av", bufs=2) as avp,
            tc.tile_pool(name="av1", bufs=1) as avp1,
            tc.tile_pool(name="st1", bufs=1) as st1,
            tc.tile_pool(name="psum", bufs=1, space="PSUM") as pp,
        ):
            # ---- resident weights / constants ----
            w_sb = {}
            for n in ("wqt", "wkt", "wvt", "wot"):
                w_sb[n] = wpool.tile([128, NCH, HID], f16, name=n + "_sb", tag=n)
            xt_r = xt.rearrange("(c p) t -> p c t", p=128)

            def load_xT(i):
                t = iop.tile([128, NCH, PAIR], f16, name=f"xT{i}", tag="xT")
                if i == 0:
                    # startup-critical: spread across 8 DMA engines
                    for c in range(NCH):
                        eng = (nc.sync, nc.scalar)[c % 2]
                        eng.dma_start(
                            t[:, c, :], xt_r[:, c, i * PAIR : (i + 1) * PAIR]
                        )
                else:
                    nc.sync.dma_start(t[:], xt_r[:, :, i * PAIR : (i + 1) * PAIR])
                return t

            xT_bufs = {0: load_xT(0)}
            k = 0
            wengs = [nc.scalar, nc.sync]
            worder = [(n, c) for c in range(NCH) for n in ("wqt", "wkt")]
            worder += [("wvt", c) for c in range(NCH)]
            worder += [("wot", c) for c in range(NCH)]
            for n, c in worder:
                wengs[k % 2].dma_start(
                    w_sb[n][:, c, :], wts[n][c * 128 : (c + 1) * 128, :]
                )
                k += 1
            ident_sb = wpool.tile([128, 128], f16, tag="ident")
            nc.scalar.dma_start(ident_sb[:], ident_d[:])
            # two static block-diag stationary buffers, zeroed once; per pair
            # only the 8 diagonal (b'=b) blocks are rewritten via DMA
            abd_bufs = []
            for j in range(2):
                ab = wpool.tile([128, NG, 8, NH, 2], f16, name=f"abds{j}",
                                tag=f"abds{j}")
                nc.vector.memset(ab[:], 0.0)
                abd_bufs.append(ab)

            # ---- per-pair state kept across pipeline stages ----
            state = {}

            def new_state(i):
                state[i] = {
                    "xT": xT_bufs.pop(i),
                    "q": {}, "k": {},
                    "comb2": iop.tile([128, NH, 80, 2], f16, name=f"c2_{i}",
                                      tag="comb2"),
                    "comb_k2": avp.tile([128, NG, 80, 2], f16, name=f"ck2_{i}",
                                        tag="comb_k2"),
                }

            def proj(i, T, which):
                """Project tile (i,T). which in ('qk', 'v')."""
                st = state[i]
                xT = st["xT"]
                if which == "qk":
                    ps_q = pp.tile([128, HID], f32, name=f"psq{i}_{T}", tag="psA")
                    ps_k = pp.tile([128, HID], f32, name=f"psk{i}_{T}", tag="psB")
                    for c in range(NCH):
                        stat = xT[:, c, T * TILE : (T + 1) * TILE]
                        for n, ps in (("wqt", ps_q), ("wkt", ps_k)):
                            for h in range(2):
                                nc.tensor.matmul(
                                    ps[:, h * 512 : (h + 1) * 512],
                                    stat,
                                    w_sb[n][:, c, h * 512 : (h + 1) * 512],
                                    start=(c == 0),
                                    stop=(c == NCH - 1),
                                )
                    q_sb = qkp.tile([128, NH, HD], f16, name=f"q{i}_{T}", tag=f"q{T}")
                    k_sb = qkp.tile([128, NH, HD], f16, name=f"k{i}_{T}", tag=f"k{T}")
                    nc.scalar.copy(q_sb[:].rearrange("p h d -> p (h d)"), ps_q[:])
                    nc.scalar.copy(k_sb[:].rearrange("p h d -> p (h d)"), ps_k[:])
                    st["q"][T] = q_sb
                    st["k"][T] = k_sb
                else:
                    ps_v = pp.tile([128, HID], f32, name=f"psv{i}_{T}", tag="psC")
                    for c in range(NCH):
                        stat = xT[:, c, T * TILE : (T + 1) * TILE]
                        for h in range(2):
                            nc.tensor.matmul(
                                ps_v[:, h * 512 : (h + 1) * 512],
                                stat,
                                w_sb["wvt"][:, c, h * 512 : (h + 1) * 512],
                                start=(c == 0),
                                stop=(c == NCH - 1),
                            )
                    # comb2[t, g, 0:16, T] = A (later); [t, g, 16:80, T] = v
                    nc.scalar.copy(
                        st["comb2"][:, :, 16:, T],
                        ps_v[:].rearrange("p (g d) -> p g d", g=NH),
                    )

            def prod_op(i, T):
                """DVE product for tile (i,T): prod[t, h, g, d] (h-major so the
                softmax g-sum reduces a contiguous inner axis)."""
                st = state[i]
                q_sb, k_sb = st["q"][T], st["k"][T]
                prod = st1.tile([128, NH, NH, HD], f16, name=f"pr{i}{T}", tag="prod")
                q_ap = q_sb[:].unsqueeze(2).broadcast_to((128, NH, NH, HD))
                k_ap = k_sb[:].unsqueeze(1).broadcast_to((128, NH, NH, HD))
                nc.vector.tensor_tensor(prod[:], k_ap, q_ap, op=OP.mult)
                st["prod"] = prod

            def tree(i, T):
                """DVE tree levels 1-4 over d; level-4 output parks in the
                pair-shared scrB2 so levels 5-6 run pair-merged."""
                st = state[i]
                p3 = st["prod"][:].rearrange("p h g d -> p (h g) d")
                scrA = st1.tile([128, NH * NH, 32], f16, tag="scrA")
                scrB = st1.tile([128, NH * NH, 16], f16, tag="scrB")
                if T == 0:
                    st["scrB2"] = scp.tile([128, 2, NH * NH, 4], f16,
                                           name=f"sb2_{i}", tag="scrB2")
                with nc.allow_low_precision(reason="fp16 score partials"):
                    nc.vector.tensor_tensor(
                        scrA[:], p3[:, :, 0:32], p3[:, :, 32:64], op=OP.add
                    )
                    nc.vector.tensor_tensor(
                        scrB[:], scrA[:, :, 0:16], scrA[:, :, 16:32], op=OP.add
                    )
                    nc.vector.tensor_tensor(
                        scrA[:, :, 0:8], scrB[:, :, 0:8], scrB[:, :, 8:16], op=OP.add
                    )
                    nc.vector.tensor_tensor(
                        st["scrB2"][:, T, :, :], scrA[:, :, 0:4], scrA[:, :, 4:8],
                        op=OP.add,
                    )

            def tail5(i):
                """Pair-merged tree levels 5-6 (both tiles in one op each)."""
                st = state[i]
                scrB2 = st["scrB2"]
                scrA5 = scp.tile([128, 2, NH * NH, 2], f16, tag="scrA5")
                scores2 = scp.tile([128, 2, NH * NH], f16, name=f"s{i}",
                                   tag="scores2")
                with nc.allow_low_precision(reason="fp16 score partials"):
                    nc.vector.tensor_tensor(
                        scrA5[:], scrB2[:, :, :, 0:2], scrB2[:, :, :, 2:4],
                        op=OP.add,
                    )
                    nc.vector.tensor_tensor(
                        scores2[:].unsqueeze(3),
                        scrA5[:, :, :, 0:1],
                        scrA5[:, :, :, 1:2],
                        op=OP.add,
                    )
                st["scores2"] = scores2

            def exp_op(i):
                st = state[i]
                ex2 = scp.tile([128, 2, NH * NH], f16, name=f"ex{i}", tag="ex2")
                nc.scalar.activation(ex2[:], st["scores2"][:], AF.Exp, scale=0.125)
                st["ex2"] = ex2

            def softmax_tail(i):
                """Pair-merged DVE ssum-reduce + recip + A-normalize,
                positioned after the next product so nothing head-blocks."""
                st = state[i]
                ex2 = st["ex2"]
                ssum2 = scp.tile([128, 2, NH], f32, tag="ssum2")
                ex_hg = ex2[:].rearrange("p T (h g) -> p T h g", h=NH)
                nc.vector.tensor_reduce(ssum2[:], ex_hg, axis=AX.X, op=OP.add)
                rs2 = scp.tile([128, 2, NH], f32, tag="rs2")
                nc.vector.reciprocal(
                    rs2[:].rearrange("p T h -> p (T h)"),
                    ssum2[:].rearrange("p T h -> p (T h)"),
                )
                # A into comb2 A-slots (both tiles):
                # comb2[t, g, h, T] = ex2[t, T, (h,g)] * rs2[t, T, h]
                nc.vector.tensor_tensor(
                    st["comb2"][:, :, 0:16, :],
                    ex2[:].rearrange("p T (h g) -> p g h T", h=NH),
                    rs2[:]
                    .rearrange("p T h -> p h T")
                    .unsqueeze(1)
                    .broadcast_to((128, NH, NH, 2)),
                    op=OP.mult,
                )

            def comb_scatter(i):
                """comb2 -> comb_k2: (b,g)-partition layout, both tiles per DMA
                (320B contiguous runs)."""
                st = state[i]
                comb2, comb_k2 = st["comb2"], st["comb_k2"]
                for grp in range(NG):
                    eng = (nc.sync, nc.scalar)[grp % 2]
                    eng.dma_start(
                        comb_k2[:, grp, :, :],
                        comb2[grp * 8 : (grp + 1) * 8, :, :, :],
                    )

            def abd(i):
                """Overwrite the 8 diagonal blocks of the static block-diag
                stationary with this pair's A values (sbuf->sbuf DMAs; engines
                can't start at partition offset 16)."""
                st = state[i]
                ab = abd_bufs[i % 2]
                for b in range(8):
                    eng = (nc.sync, nc.scalar)[b % 2]
                    eng.dma_start(
                        ab[b * 16 : (b + 1) * 16, :, b, :, :],
                        st["comb_k2"][b * 16 : (b + 1) * 16, :, 0:16, :],
                    )
                st["abd2"] = ab

            def av(e, T):
                """AV matmuls for tile (e,T) -> attn_pm2[..., T]."""
                st = state[e]
                abd2, comb_k2 = st["abd2"], st["comb_k2"]
                if T == 0:
                    st["attn_pm2"] = avp1.tile(
                        [128, NH, HD, 2], f16, name=f"apm{e}", tag="attn_pm2"
                    )
                pa = [
                    pp.tile([128, NG // 2, HD], f32, name=f"pa{e}{T}{h}",
                            tag=f"pav{h}")
                    for h in range(2)
                ]
                for grp in range(NG):
                    nc.tensor.matmul(
                        pa[grp // 8][:, grp % 8, :],
                        abd2[:, grp, :, :, T].rearrange("p b h -> p (b h)"),
                        comb_k2[:, grp, 16:, T],
                        start=True,
                        stop=True,
                    )
                for h in range(2):
                    nc.scalar.copy(
                        st["attn_pm2"][:, h * 8 : (h + 1) * 8, :, T], pa[h][:]
                    )

            def attn_scatter(e):
                st = state[e]
                st["attn16_2"] = avp1.tile(
                    [128, NH, HD, 2], f16, name=f"a16{e}", tag="attn16_2"
                )
                for grp in range(NG):
                    eng = (nc.scalar, nc.sync)[grp % 2]
                    eng.dma_start(
                        st["attn16_2"][grp * 8 : (grp + 1) * 8, :, :, :],
                        st["attn_pm2"][:, grp, :, :],
                    )

            def trans_wo2(e):
                """PE transposes + output projection for BOTH tiles, ordered
                so the PE never waits on an oT copy: transposes(T0)->pav0,
                transposes(T1)->pav1 (runs while ACT copies T0's), then the
                wo matmuls land in the slot freed by the matching copy."""
                st = state[e]
                st["y_sb2"] = avp1.tile(
                    [128, 2, HID], f16, name=f"ysb{e}", tag="y_sb2"
                )
                oTs = {}
                for T in range(2):
                    oT = avp1.tile([128, NCH, TILE], f16, name=f"oT{e}{T}",
                                   tag=f"oT{T}")
                    pt = pp.tile([128, NCH * TILE], f16, name=f"pt{e}{T}",
                                 tag=f"pav{T}")
                    for c in range(NCH):
                        nc.tensor.transpose(
                            pt[:, c * 128 : (c + 1) * 128],
                            st["attn16_2"][:, 2 * c : 2 * c + 2, :, T].rearrange(
                                "p h d -> p (h d)"
                            ),
                            ident_sb[:],
                        )
                    nc.scalar.copy(oT[:].rearrange("p c t -> p (c t)"), pt[:])
                    oTs[T] = oT
                for T in range(2):
                    py = [
                        pp.tile([128, 512], f32, name=f"py{e}{T}{h}",
                                tag=f"pav{h}")
                        for h in range(2)
                    ]
                    for c in range(NCH):
                        for h in range(2):
                            nc.tensor.matmul(
                                py[h][:],
                                oTs[T][:, c, :],
                                w_sb["wot"][:, c, h * 512 : (h + 1) * 512],
                                start=(c == 0),
                                stop=(c == NCH - 1),
                            )
                    for h in range(2):
                        nc.scalar.copy(
                            st["y_sb2"][:, T, h * 512 : (h + 1) * 512], py[h][:]
                        )

            def y_out(e):
                st = state[e]
                t0 = e * PAIR
                ydst = y[t0 : t0 + PAIR, :].rearrange("(T t) f -> t T f", T=2)
                if e == NP - 1:
                    # drain-critical: spread across 4 DMA engines
                    for j in range(4):
                        eng = (nc.sync, nc.scalar)[j % 2]
                        eng.dma_start(
                            ydst[:, :, j * 256 : (j + 1) * 256],
                            st["y_sb2"][:, :, j * 256 : (j + 1) * 256],
                        )
                else:
                    nc.sync.dma_start(ydst, st["y_sb2"][:])

            # ---- main loop: projections ONE pair ahead, AV one pair behind --
            # iteration i: proj(i+1), DVE scores(i), AV/output chain(i-1)
            new_state(0)
            proj(0, 0, "qk")
            proj(0, 0, "v")
            proj(0, 1, "qk")
            proj(0, 1, "v")
            for i in range(NP + 1):
                e = i - 1
                if i + 1 < NP:
                    xT_bufs[i + 1] = load_xT(i + 1)
                    new_state(i + 1)
                    proj(i + 1, 0, "qk")
                if i < NP:
                    prod_op(i, 0)
                if e >= 0:
                    softmax_tail(e)     # deferred: sits after prod(i,T0)
                    comb_scatter(e)
                    abd(e)
                if i + 1 < NP:
                    proj(i + 1, 0, "v")
                    proj(i + 1, 1, "qk")
                if i < NP:
                    tree(i, 0)
                if e >= 0:
                    av(e, 0)
                    av(e, 1)
                    attn_scatter(e)
                if i < NP:
                    prod_op(i, 1)
                if i + 1 < NP:
                    proj(i + 1, 1, "v")
                if e >= 0:
                    trans_wo2(e)
                    y_out(e)
                    del state[e]
                if i < NP:
                    tree(i, 1)
                    tail5(i)
                    exp_op(i)

    nc.compile()
    _cache["nc"] = nc


def _prep_inputs(x, wq, wk, wv, wo):
    x2 = np.asarray(x, dtype=np.float32).reshape(-1, HID)
    w16 = {
        n: np.ascontiguousarray(np.asarray(w, dtype=np.float32).T).astype(np.float16)
        for n, w in (("wqt", wq), ("wkt", wk), ("wvt", wv), ("wot", wo))
    }
    in_maps = []
    for i in range(N_CORES):
        sh = x2[i * TPC : (i + 1) * TPC].astype(np.float16)
        m = {"xt": np.ascontiguousarray(sh.T),
             "ident": np.eye(128, dtype=np.float16)}
        m.update(w16)
        in_maps.append(m)
    return in_maps


def kernel(x, wq, wk, wv, wo, _trace=False):
    from concourse import bass_utils

    _build()
    in_maps = _prep_inputs(x, wq, wk, wv, wo)
    res = bass_utils.run_bass_kernel_spmd(
        _cache["nc"], in_maps, core_ids=list(range(N_CORES)), trace=_trace
    )
    kernel.last_result = res
    B, S = 4, 4096
    out = np.concatenate([r["y"] for r in res.results], axis=0)
    return out.reshape(B, S, HID).astype(np.float32)
